# revision 16
# baseline (speedup 1.0000x reference)
"""Trainium2 Bass kernel for an (unscaled-softmax) attention block.

Problem: x:[4,2048,1024] f32, wq/wk/wv:[1024,1024] f32
    q = x@wq; k = x@wk; v = x@wv
    out = softmax(q @ k^T, axis=-1) @ v        (NO 1/sqrt(d) scaling)

Algebraic restructure: scores = q@k^T = x @ (wq wk^T) @ x^T = (x@M) @ x^T
with M = wq wk^T, so the q- and k-projections (and the baseline's k^T
pair exchange) collapse into a single y = x@M projection whose score
rhs (raw x^T) comes straight from the host. M is computed on device
(sharded 128 rows/core) and AllGather'd across all 8 cores -- the only
collective in the kernel; it hides under the v-projection.

Sharding: 8 cores = 4 batches x 2 query-halves. Each core computes y
for its OWN 1024 rows and v for the full 2048 rows of its batch (v is
recomputed rather than exchanged: +27us of TensorE beats a pair-wise
AllGather plus its DRAM round trip and scheduling hazards). All row
indexing is in LOCAL order (own half first), which keeps the single
program uniform across cores.

Precision: the unscaled scores are ~N(0, 32768^2) and the minimum
top-2 gap over this input set is ~2.7, so the score path (M, y,
scores) needs ~17-bit accuracy. It uses a bf16x2 split: a = hi(a) +
lo(a), a@b ~= ah@bh + ah@bl + al@bh -- three full-speed bf16 matmuls
with fp32 PSUM accumulation. (Native fp32 matmuls were measured
FASTER per-flop than the triple -- 515ns vs 3x259ns at N=512 -- but a
sustained 8-core fp32 mix deterministically trips the chip power
throttle to K=13/16, slowing the WHOLE kernel ~25%; bf16-dense runs
stay at full clock.) The v / attention@v path is plain bf16.

Scheduling notes (from trace analysis):
- ~48 warmup matmuls on a zeroed tile run during the initial DMA
  window so the PE's HAM clock gate is warm (2.4GHz) when real work
  starts; cold-start otherwise costs 2x for the first phases.
- Weights and x^T stream in large unit DMAs striped across both HWDGE
  queues (sync+scalar); per-DMA dispatch on an engine costs ~0.6us,
  so many small just-in-time DMAs throttle the PE (this was the
  dominant stall in earlier versions).
- The gathered M lands in SBUF via ONE rearranged 4MB DMA.
- The softmax-weight transpose runs on the DMA xbar (one instruction
  per q-tile), not the PE.
"""

import numpy as np

import concourse.bass as bass
import concourse.bacc as bacc
import concourse.tile as tile
from concourse import mybir

F32 = mybir.dt.float32
BF16 = mybir.dt.bfloat16
P = 128


def build_attention(SQ=1024, T=2048, D=1024, ncores=8):
    """Build the single-core Bass program (uniform across all cores).

    Per-core inputs (all layout/precision prep happens on the host):
      xsp [DT, P, 2, T]  x^T of the pair batch in LOCAL row order (own
                         query half first), bf16 hi/lo
      wqt [DT, P, 2, P]  wq^T columns for this core's M row-tile, hi/lo
      wkt [DT, P, 2, D]  full wk^T, hi/lo
      wvb [DT, P, D]     wv rows, bf16
    out: [SQ, D] f32 (own query rows)
    """
    CH = 512             # psum chunk (max moving free dim)
    assert SQ % P == 0 and T % P == 0 and D % P == 0
    DT = D // P          # contraction / d_out tiles
    TT = T // P          # t tiles
    QT = SQ // P         # q row tiles
    TC = T // CH         # score chunks per q-tile
    DC = D // CH         # out-dim chunks
    SC = SQ // CH        # own-row chunks

    nc = bacc.Bacc(
        "TRN2", target_bir_lowering=False, debug=False, num_devices=ncores
    )
    xsp_d = nc.dram_tensor("xsp", [DT, P, 2, T], BF16, kind="ExternalInput")
    wqt_d = nc.dram_tensor("wqt", [DT, P, 2, P], BF16, kind="ExternalInput")
    wkt_d = nc.dram_tensor("wkt", [DT, P, 2, D], BF16, kind="ExternalInput")
    wv_d = nc.dram_tensor("wvb", [DT, P, D], BF16, kind="ExternalInput")
    out_d = nc.dram_tensor("out", [SQ, D], F32, kind="ExternalOutput")

    from contextlib import ExitStack

    with tile.TileContext(nc) as tc, ExitStack() as ctx:
        # Persistent SBUF tensors (whole-kernel lifetime).
        arena = ctx.enter_context(tc.tile_pool(name="arena", bufs=1))
        xsp_u = [arena.tile([P, 2, T], BF16, tag=f"xs{d}", name=f"xs{d}") for d in range(DT)]
        qu = [arena.tile([P, 2, SQ], BF16, tag=f"q{m}", name=f"q{m}") for m in range(DT)]
        vpair = [arena.tile([P, 2, D], BF16, tag=f"vp{i}", name=f"vp{i}") for i in range(TT // 2)]
        v_sb = [vpair[t // 2][:, t % 2, :] for t in range(TT)]

        def split_psum(ps, hi_dst, lo_dst):
            """hi = bf16(ps); lo = bf16(ps - hi)  (fp32 internally)."""
            nc.vector.tensor_copy(hi_dst, ps)
            nc.vector.tensor_sub(lo_dst, ps, hi_dst)

        # Collective bounce buffers (internal DRAM, dep-tracked).
        p_cc = ctx.enter_context(tc.tile_pool(name="cc", bufs=1, space="DRAM"))
        cc_m_in = p_cc.tile([P, 2, D], BF16, tag="cmi", name="cmi")
        cc_m_out = p_cc.tile([ncores, P, 2, D], BF16, tag="cmo", name="cmo")
        all_group = [list(range(ncores))]

        # ---- warmup: keep the PE busy while the first weights stream in,
        # so the HAM clock gate reaches 8/8 before phase A ----
        with (
            tc.tile_pool(name="wrm", bufs=1) as p_wrm,
            tc.tile_pool(name="wps", bufs=1, space="PSUM") as p_wps,
        ):
            wz = p_wrm.tile([P, CH], BF16, tag="wz")
            nc.vector.memset(wz, 0)
            # two accumulation groups on alternating banks: consecutive
            # same-bank matmuls serialize on the drain, alternating pipelines
            wpss = [
                p_wps.tile([P, CH], F32, tag=f"wps{j}", name=f"wps{j}")
                for j in range(2)
            ]
            NW = 16
            for r in range(NW):
                nc.tensor.matmul(
                    wpss[r % 2], wz[:, :P], wz, start=(r < 2), stop=(r >= NW - 2)
                )

        # ---- phase A: M-rows = wq^T-slice^T @ wk^T (triple bf16), then one
        # 8-wide AllGather. Weight units stream striped across both queues. --
        with (
            tc.tile_pool(name="wqp", bufs=1) as p_wq,
            tc.tile_pool(name="wkp", bufs=1) as p_wk,
            tc.tile_pool(name="mst", bufs=2) as p_mst,
            tc.tile_pool(name="aps", bufs=1, space="PSUM") as p_aps,
        ):
            # jj=0's operands load at the very head of both queues so the
            # first real matmul can issue ~6us in; later units stream behind
            wqts, wkts = [], []
            for jj in range(DT):
                wq_t = p_wq.tile([P, 2, P], BF16, tag=f"wq{jj}", name=f"wq{jj}")
                wk_t = p_wk.tile([P, 2, D], BF16, tag=f"wk{jj}", name=f"wk{jj}")
                nc.sync.dma_start(out=wq_t, in_=wqt_d[jj])
                eng = nc.scalar if jj % 2 == 0 else nc.sync
                eng.dma_start(out=wk_t, in_=wkt_d[jj])
                wqts.append(wq_t)
                wkts.append(wk_t)
            # both column-halves accumulate together with the ch loop INNER,
            # so consecutive matmuls alternate PSUM banks and pipeline
            pss_a = [
                p_aps.tile([P, CH], F32, tag=f"aps{ch}", name=f"aps{ch}")
                for ch in range(2)
            ]
            for jj in range(DT):
                for ap, bp in ((0, 0), (0, 1), (1, 0)):
                    for ch in range(2):
                        nc.tensor.matmul(
                            pss_a[ch],
                            wqts[jj][:, ap, :],
                            wkts[jj][:, bp, ch * CH : (ch + 1) * CH],
                            start=(jj == 0 and ap == 0 and bp == 0),
                            stop=(jj == DT - 1 and ap == 1),
                        )
            for ch in range(2):
                mst = p_mst.tile([P, 2, CH], BF16, tag="mst")
                split_psum(pss_a[ch], mst[:, 0, :], mst[:, 1, :])
                nc.gpsimd.dma_start(
                    out=cc_m_in[:, :, ch * CH : (ch + 1) * CH], in_=mst
                )
            nc.gpsimd.collective_compute(
                "AllGather",
                mybir.AluOpType.bypass,
                replica_groups=all_group,
                ins=[cc_m_in[:]],
                outs=[cc_m_out[:]],
            )

        # ---- x^T and wv loads: wv as one 2MB DMA at the head of the
        # scalar stream (phase B's gating input), x^T as 8 whole-unit 1MB
        # DMAs (4KB-contiguous rows, minimal dispatch) striped across both
        # queues. Fewer, larger DMAs measure ~2x the effective bandwidth of
        # t-sliced streams and close the A->B TensorE gap. ----
        es_b = ExitStack()
        p_wv = es_b.enter_context(tc.tile_pool(name="wvp", bufs=1))
        wv_big = p_wv.tile([P, DT, D], BF16, tag="wvb", name="wvb")
        nc.scalar.dma_start(
            out=wv_big[:], in_=wv_d[:].rearrange("k p d -> p k d")
        )
        wv_bf = [wv_big[:, kk, :] for kk in range(DT)]
        for d in range(DT):
            eng = nc.sync if d % 2 == 0 else nc.scalar
            eng.dma_start(out=xsp_u[d][:], in_=xsp_d[d])

        # ---- phase B: v = x @ wv for the FULL pair batch (bf16, local
        # order); hides the M AllGather ----
        with tc.tile_pool(name="vps", bufs=2, space="PSUM") as p_vps:
            for t in range(TT):
                pss = [
                    p_vps.tile([P, CH], F32, tag=f"vps{n}", name=f"vps{n}")
                    for n in range(DC)
                ]
                for kk in range(DT):
                    lhs = xsp_u[kk][:, 0, t * P : (t + 1) * P]
                    for n in range(DC):
                        nc.tensor.matmul(
                            pss[n],
                            lhs,
                            wv_bf[kk][:, n * CH : (n + 1) * CH],
                            start=(kk == 0),
                            stop=(kk == DT - 1),
                        )
                for n in range(DC):
                    nc.vector.tensor_copy(
                        v_sb[t][:, n * CH : (n + 1) * CH], pss[n]
                    )
        es_b.close()

        # ---- phase C: y^T = M-tiles^T @ x^T-own (triple bf16). The whole
        # gathered M lands in SBUF via one rearranged 4MB DMA. ----
        with (
            tc.tile_pool(name="mu", bufs=1) as p_mu,
            tc.tile_pool(name="pps", bufs=2, space="PSUM") as p_pps,
        ):
            mu = p_mu.tile([P, DT, 2, D], BF16, tag="mu", name="mu")
            for k in range(DT):
                nc.sync.dma_start(out=mu[:, k, :, :], in_=cc_m_out[k])
            for m in range(DT):
                pss = [
                    p_pps.tile([P, CH], F32, tag=f"pps{c}", name=f"pps{c}")
                    for c in range(SC)
                ]
                for kk in range(DT):
                    for mp, xp in ((0, 0), (0, 1), (1, 0)):
                        for c in range(SC):
                            nc.tensor.matmul(
                                pss[c],
                                mu[:, kk, mp, m * P : (m + 1) * P],
                                xsp_u[kk][:, xp, c * CH : (c + 1) * CH],
                                start=(kk == 0 and mp == 0 and xp == 0),
                                stop=(kk == DT - 1 and mp == 1),
                            )
                for c in range(SC):
                    split_psum(
                        pss[c],
                        qu[m][:, 0, c * CH : (c + 1) * CH],
                        qu[m][:, 1, c * CH : (c + 1) * CH],
                    )

        # ---- phase D: per q-tile attention, one-stage software pipeline:
        # PE runs scores(qi), then AV of qi-1 while ACT exponentiates qi and
        # the DMA xbar transposes qi's softmax weights. ----
        with (
            tc.tile_pool(name="stats", bufs=4) as p_st,
            tc.tile_pool(name="ssb", bufs=2) as p_ssb,
            tc.tile_pool(name="exps", bufs=2) as p_ex,
            tc.tile_pool(name="wtsb", bufs=2) as p_wtsb,
            tc.tile_pool(name="osb", bufs=2) as p_o,
            tc.tile_pool(name="scps", bufs=1, space="PSUM") as p_sc,
            tc.tile_pool(name="avps", bufs=1, space="PSUM") as p_av,
        ):

            def emit_scores(qi):
                ssb = p_ssb.tile([P, T], F32, tag="ssb")
                for c in range(TC):
                    scs[c] = p_sc.tile([P, CH], F32, tag=f"sc{c}", name=f"sc{c}")
                for kk in range(DT):
                    for qp, kp in ((0, 0), (0, 1), (1, 0)):
                        lhs = qu[kk][:, qp, qi * P : (qi + 1) * P]
                        for c in range(TC):
                            nc.tensor.matmul(
                                scs[c],
                                lhs,
                                xsp_u[kk][:, kp, c * CH : (c + 1) * CH],
                                start=(kk == 0 and qp == 0 and kp == 0),
                                stop=(kk == DT - 1 and qp == 1),
                            )
                for c in range(TC):
                    nc.vector.tensor_copy(
                        ssb[:, c * CH : (c + 1) * CH], scs[c]
                    )
                return ssb

            def emit_softmax(qi, ssb):
                mx4 = p_st.tile([P, TC], F32, tag="mx4")
                for c in range(TC):
                    nc.vector.reduce_max(
                        mx4[:, c : c + 1],
                        ssb[:, c * CH : (c + 1) * CH],
                        axis=mybir.AxisListType.X,
                    )
                negmx = p_st.tile([P, 1], F32, tag="negmx")
                mx = p_st.tile([P, 1], F32, tag="mx")
                nc.vector.reduce_max(mx, mx4, axis=mybir.AxisListType.X)
                nc.scalar.mul(negmx, mx, -1.0)
                sums = p_st.tile([P, TC], F32, tag="sums")
                exps = p_ex.tile([P, T], BF16, tag="exps")
                for c in range(TC):
                    nc.scalar.activation(
                        out=exps[:, c * CH : (c + 1) * CH],
                        in_=ssb[:, c * CH : (c + 1) * CH],
                        func=mybir.ActivationFunctionType.Exp,
                        bias=negmx[:, 0:1],
                        scale=1.0,
                        accum_out=sums[:, c : c + 1],
                    )
                ssum = p_st.tile([P, 1], F32, tag="ssum")
                nc.vector.reduce_sum(ssum, sums, axis=mybir.AxisListType.X)
                rsum = p_st.tile([P, 1], F32, tag="rsum")
                nc.vector.reciprocal(rsum, ssum)
                # full [s, t] -> [t, s] transpose on the DMA xbar:
                # wt[p, k, j] = exps[j, k*P + p]
                wt = p_wtsb.tile([P, TT, P], BF16, tag="wt")
                nc.sync.dma_start_transpose(wt[:], exps[:])
                return wt, rsum

            def emit_av(qi, wt, rsum):
                avs = [
                    p_av.tile([P, CH], F32, tag=f"av{n}", name=f"av{n}")
                    for n in range(DC)
                ]
                for t in range(TT):
                    lhs = wt[:, t, :]
                    for n in range(DC):
                        nc.tensor.matmul(
                            avs[n],
                            lhs,
                            v_sb[t][:, n * CH : (n + 1) * CH],
                            start=(t == 0),
                            stop=(t == TT - 1),
                        )
                osb = p_o.tile([P, D], F32, tag="o")
                for n in range(DC):
                    nc.vector.tensor_scalar_mul(
                        osb[:, n * CH : (n + 1) * CH], avs[n], rsum[:, 0:1]
                    )
                nc.scalar.dma_start(out=out_d[qi * P : (qi + 1) * P, :], in_=osb)

            scs = [None] * TC
            prev = None
            for qi in range(QT):
                ssb = emit_scores(qi)
                if prev is not None:
                    emit_av(*prev)
                wt, rsum = emit_softmax(qi, ssb)
                prev = (qi, wt, rsum)
            emit_av(*prev)

    nc.compile()
    return nc


_CACHE = {}


def _built_full():
    if "nc" not in _CACHE:
        _CACHE["nc"] = build_attention(1024, 2048, 1024)
    return _CACHE["nc"]


def _bf16_split(a):
    """fp32 array -> (hi, lo) bf16 with hi + lo ~= a (RNE, matches DVE)."""
    import ml_dtypes

    hi = a.astype(ml_dtypes.bfloat16)
    lo = (a - hi.astype(np.float32)).astype(ml_dtypes.bfloat16)
    return hi, lo


def host_prep_x(x_rows, P=128):
    """x rows [XR, D] f32 -> [DT, P, 2, XR] bf16 (x^T per d-tile, split)."""
    XR, D = x_rows.shape
    xT = np.ascontiguousarray(x_rows.T.astype(np.float32))  # [D, XR]
    hi, lo = _bf16_split(xT)
    out = np.stack([hi, lo], axis=1).reshape(D // P, P, 2, XR)
    return np.ascontiguousarray(out)


def host_prep_wT(w, c0=None, c1=None, P=128):
    """w [D, D] f32 -> w^T cols [c0:c1] as [DT, P, 2, c1-c0] bf16 hi/lo."""
    D = w.shape[0]
    wT = np.ascontiguousarray(w.astype(np.float32).T)  # [j, a]
    if c0 is not None:
        wT = wT[:, c0:c1]
    hi, lo = _bf16_split(wT)
    out = np.stack([hi, lo], axis=1).reshape(D // P, P, 2, wT.shape[1])
    return np.ascontiguousarray(out)


def host_prep_wv(wv, P=128):
    import ml_dtypes

    D = wv.shape[0]
    return np.ascontiguousarray(
        wv.astype(np.float32).astype(ml_dtypes.bfloat16).reshape(D // P, P, D)
    )


def _make_in_maps(x, wq, wk, wv):
    """Per-core input maps: core c = (batch c//2, query-half c%2)."""
    x = np.ascontiguousarray(np.asarray(x, dtype=np.float32))
    wq = np.asarray(wq, dtype=np.float32)
    wk = np.asarray(wk, dtype=np.float32)
    wv = np.asarray(wv, dtype=np.float32)
    B, S, D = x.shape
    half = S // 2
    wkt = host_prep_wT(wk)
    wvb = host_prep_wv(wv)
    in_maps = []
    for c in range(8):
        b, h = divmod(c, 2)
        if h == 0:
            xloc = x[b]
        else:
            xloc = np.concatenate([x[b][half:], x[b][:half]], axis=0)
        in_maps.append(
            {
                "xsp": host_prep_x(xloc),
                "wqt": host_prep_wT(wq, c * P, (c + 1) * P),
                "wkt": wkt,
                "wvb": wvb,
            }
        )
    return in_maps, (B, S, D)


def _assemble(results, shape):
    B, S, D = shape
    half = S // 2
    out = np.empty((B, S, D), np.float32)
    for c in range(8):
        b, h = divmod(c, 2)
        out[b, h * half : (h + 1) * half] = results[c]["out"]
    return out


def kernel(x, wq, wk, wv):
    """Full (unsharded) inputs -> full output, running SPMD on 8 cores."""
    from concourse.bass_utils import run_bass_kernel_spmd

    in_maps, shape = _make_in_maps(x, wq, wk, wv)
    nc = _built_full()
    res = run_bass_kernel_spmd(nc, in_maps, core_ids=list(range(8))).results
    return _assemble(res, shape)


# revision 17
# speedup vs baseline: 1.0063x; 1.0063x over previous
"""Trainium2 Bass kernel for an (unscaled-softmax) attention block.

Problem: x:[4,2048,1024] f32, wq/wk/wv:[1024,1024] f32
    q = x@wq; k = x@wk; v = x@wv
    out = softmax(q @ k^T, axis=-1) @ v        (NO 1/sqrt(d) scaling)

Algebraic restructure: scores = q@k^T = x @ (wq wk^T) @ x^T = (x@M) @ x^T
with M = wq wk^T, so the q- and k-projections (and the baseline's k^T
pair exchange) collapse into a single y = x@M projection whose score
rhs (raw x^T) comes straight from the host. M is computed on device
(sharded 128 rows/core) and AllGather'd across all 8 cores -- the only
collective in the kernel; it hides under the v-projection.

Sharding: 8 cores = 4 batches x 2 query-halves. Each core computes y
for its OWN 1024 rows and v for the full 2048 rows of its batch (v is
recomputed rather than exchanged: +27us of TensorE beats a pair-wise
AllGather plus its DRAM round trip and scheduling hazards). All row
indexing is in LOCAL order (own half first), which keeps the single
program uniform across cores.

Precision: the unscaled scores are ~N(0, 32768^2) and the minimum
top-2 gap over this input set is ~2.7, so the score path (M, y,
scores) needs ~17-bit accuracy. It uses a bf16x2 split: a = hi(a) +
lo(a), a@b ~= ah@bh + ah@bl + al@bh -- three full-speed bf16 matmuls
with fp32 PSUM accumulation. (Native fp32 matmuls were measured
FASTER per-flop than the triple -- 515ns vs 3x259ns at N=512 -- but a
sustained 8-core fp32 mix deterministically trips the chip power
throttle to K=13/16, slowing the WHOLE kernel ~25%; bf16-dense runs
stay at full clock.) The v / attention@v path is plain bf16.

Scheduling notes (from trace analysis):
- ~48 warmup matmuls on a zeroed tile run during the initial DMA
  window so the PE's HAM clock gate is warm (2.4GHz) when real work
  starts; cold-start otherwise costs 2x for the first phases.
- Weights and x^T stream in large unit DMAs striped across both HWDGE
  queues (sync+scalar); per-DMA dispatch on an engine costs ~0.6us,
  so many small just-in-time DMAs throttle the PE (this was the
  dominant stall in earlier versions).
- The gathered M lands in SBUF via ONE rearranged 4MB DMA.
- The softmax-weight transpose runs on the DMA xbar (one instruction
  per q-tile), not the PE.
"""

import numpy as np

import concourse.bass as bass
import concourse.bacc as bacc
import concourse.tile as tile
from concourse import mybir

F32 = mybir.dt.float32
BF16 = mybir.dt.bfloat16
P = 128


def build_attention(SQ=1024, T=2048, D=1024, ncores=8):
    """Build the single-core Bass program (uniform across all cores).

    Per-core inputs (all layout/precision prep happens on the host):
      xsp [DT, P, 2, T]  x^T of the pair batch in LOCAL row order (own
                         query half first), bf16 hi/lo
      wqt [DT, P, 2, P]  wq^T columns for this core's M row-tile, hi/lo
      wkt [DT, P, 2, D]  full wk^T, hi/lo
      wvb [DT, P, D]     wv rows, bf16
    out: [SQ, D] f32 (own query rows)
    """
    CH = 512             # psum chunk (max moving free dim)
    assert SQ % P == 0 and T % P == 0 and D % P == 0
    DT = D // P          # contraction / d_out tiles
    TT = T // P          # t tiles
    QT = SQ // P         # q row tiles
    TC = T // CH         # score chunks per q-tile
    DC = D // CH         # out-dim chunks
    SC = SQ // CH        # own-row chunks

    nc = bacc.Bacc(
        "TRN2", target_bir_lowering=False, debug=False, num_devices=ncores
    )
    xsp_d = nc.dram_tensor("xsp", [DT, P, 2, T], BF16, kind="ExternalInput")
    wqt_d = nc.dram_tensor("wqt", [DT, P, 2, P], BF16, kind="ExternalInput")
    wkt_d = nc.dram_tensor("wkt", [DT, P, 2, D], BF16, kind="ExternalInput")
    wv_d = nc.dram_tensor("wvb", [DT, P, D], BF16, kind="ExternalInput")
    out_d = nc.dram_tensor("out", [SQ, D], F32, kind="ExternalOutput")

    from contextlib import ExitStack

    with tile.TileContext(nc) as tc, ExitStack() as ctx:
        # Persistent SBUF tensors (whole-kernel lifetime).
        arena = ctx.enter_context(tc.tile_pool(name="arena", bufs=1))
        xsp_u = [arena.tile([P, 2, T], BF16, tag=f"xs{d}", name=f"xs{d}") for d in range(DT)]
        qu = [arena.tile([P, 2, SQ], BF16, tag=f"q{m}", name=f"q{m}") for m in range(DT)]
        vpair = [arena.tile([P, 2, D], BF16, tag=f"vp{i}", name=f"vp{i}") for i in range(TT // 2)]
        v_sb = [vpair[t // 2][:, t % 2, :] for t in range(TT)]

        def split_psum(ps, hi_dst, lo_dst):
            """hi = bf16(ps); lo = bf16(ps - hi)  (fp32 internally)."""
            nc.vector.tensor_copy(hi_dst, ps)
            nc.vector.tensor_sub(lo_dst, ps, hi_dst)

        # Collective bounce buffers (internal DRAM, dep-tracked).
        p_cc = ctx.enter_context(tc.tile_pool(name="cc", bufs=1, space="DRAM"))
        cc_m_in = p_cc.tile([P, 2, D], BF16, tag="cmi", name="cmi")
        cc_m_out = p_cc.tile([ncores, P, 2, D], BF16, tag="cmo", name="cmo")
        all_group = [list(range(ncores))]

        # ---- warmup: keep the PE busy while the first weights stream in,
        # so the HAM clock gate reaches 8/8 before phase A ----
        with (
            tc.tile_pool(name="wrm", bufs=1) as p_wrm,
            tc.tile_pool(name="wps", bufs=1, space="PSUM") as p_wps,
        ):
            wz = p_wrm.tile([P, CH], BF16, tag="wz")
            nc.vector.memset(wz, 0)
            # two accumulation groups on alternating banks: consecutive
            # same-bank matmuls serialize on the drain, alternating pipelines
            wpss = [
                p_wps.tile([P, CH], F32, tag=f"wps{j}", name=f"wps{j}")
                for j in range(2)
            ]
            NW = 16
            for r in range(NW):
                nc.tensor.matmul(
                    wpss[r % 2], wz[:, :P], wz, start=(r < 2), stop=(r >= NW - 2)
                )

        # ---- phase A: M-rows = wq^T-slice^T @ wk^T (triple bf16), then one
        # 8-wide AllGather. Weight units stream striped across both queues. --
        with (
            tc.tile_pool(name="wqp", bufs=1) as p_wq,
            tc.tile_pool(name="wkp", bufs=1) as p_wk,
            tc.tile_pool(name="mst", bufs=2) as p_mst,
            tc.tile_pool(name="aps", bufs=1, space="PSUM") as p_aps,
        ):
            # jj=0's operands load at the very head of both queues so the
            # first real matmul can issue ~6us in; later units stream behind
            wqts, wkts = [], []
            for jj in range(DT):
                wq_t = p_wq.tile([P, 2, P], BF16, tag=f"wq{jj}", name=f"wq{jj}")
                wk_t = p_wk.tile([P, 2, D], BF16, tag=f"wk{jj}", name=f"wk{jj}")
                nc.sync.dma_start(out=wq_t, in_=wqt_d[jj])
                eng = nc.scalar if jj % 2 == 0 else nc.sync
                eng.dma_start(out=wk_t, in_=wkt_d[jj])
                wqts.append(wq_t)
                wkts.append(wk_t)
            # both column-halves accumulate together with the ch loop INNER,
            # so consecutive matmuls alternate PSUM banks and pipeline
            pss_a = [
                p_aps.tile([P, CH], F32, tag=f"aps{ch}", name=f"aps{ch}")
                for ch in range(2)
            ]
            for jj in range(DT):
                for ap, bp in ((0, 0), (0, 1), (1, 0)):
                    for ch in range(2):
                        nc.tensor.matmul(
                            pss_a[ch],
                            wqts[jj][:, ap, :],
                            wkts[jj][:, bp, ch * CH : (ch + 1) * CH],
                            start=(jj == 0 and ap == 0 and bp == 0),
                            stop=(jj == DT - 1 and ap == 1),
                        )
            for ch in range(2):
                mst = p_mst.tile([P, 2, CH], BF16, tag="mst")
                split_psum(pss_a[ch], mst[:, 0, :], mst[:, 1, :])
                nc.gpsimd.dma_start(
                    out=cc_m_in[:, :, ch * CH : (ch + 1) * CH], in_=mst
                )
            nc.gpsimd.collective_compute(
                "AllGather",
                mybir.AluOpType.bypass,
                replica_groups=all_group,
                ins=[cc_m_in[:]],
                outs=[cc_m_out[:]],
            )

        # ---- x^T and wv loads: wv first (phase B needs it), then x^T in
        # t-slices striped across both queues so B can start early ----
        es_b = ExitStack()
        p_wv = es_b.enter_context(tc.tile_pool(name="wvp", bufs=1))
        wv_bf = []
        for kk in range(DT):
            wvb = p_wv.tile([P, D], BF16, tag=f"wvb{kk}", name=f"wvb{kk}")
            nc.scalar.dma_start(out=wvb, in_=wv_d[kk])
            wv_bf.append(wvb)
        for sl in range(TC):
            for d in range(DT):
                eng = nc.sync if (sl * DT + d) % 2 == 0 else nc.scalar
                eng.dma_start(
                    out=xsp_u[d][:, :, sl * CH : (sl + 1) * CH],
                    in_=xsp_d[d, :, :, sl * CH : (sl + 1) * CH],
                )

        # ---- phase B: v = x @ wv for the FULL pair batch (bf16, local
        # order); hides the M AllGather ----
        with tc.tile_pool(name="vps", bufs=2, space="PSUM") as p_vps:
            for t in range(TT):
                pss = [
                    p_vps.tile([P, CH], F32, tag=f"vps{n}", name=f"vps{n}")
                    for n in range(DC)
                ]
                for kk in range(DT):
                    lhs = xsp_u[kk][:, 0, t * P : (t + 1) * P]
                    for n in range(DC):
                        nc.tensor.matmul(
                            pss[n],
                            lhs,
                            wv_bf[kk][:, n * CH : (n + 1) * CH],
                            start=(kk == 0),
                            stop=(kk == DT - 1),
                        )
                for n in range(DC):
                    nc.vector.tensor_copy(
                        v_sb[t][:, n * CH : (n + 1) * CH], pss[n]
                    )
        es_b.close()

        # ---- phase C: y^T = M-tiles^T @ x^T-own (triple bf16). The whole
        # gathered M lands in SBUF via one rearranged 4MB DMA. ----
        with (
            tc.tile_pool(name="mu", bufs=1) as p_mu,
            tc.tile_pool(name="pps", bufs=2, space="PSUM") as p_pps,
        ):
            mu = p_mu.tile([P, DT, 2, D], BF16, tag="mu", name="mu")
            for k in range(DT):
                nc.sync.dma_start(out=mu[:, k, :, :], in_=cc_m_out[k])
            for m in range(DT):
                pss = [
                    p_pps.tile([P, CH], F32, tag=f"pps{c}", name=f"pps{c}")
                    for c in range(SC)
                ]
                for kk in range(DT):
                    for mp, xp in ((0, 0), (0, 1), (1, 0)):
                        for c in range(SC):
                            nc.tensor.matmul(
                                pss[c],
                                mu[:, kk, mp, m * P : (m + 1) * P],
                                xsp_u[kk][:, xp, c * CH : (c + 1) * CH],
                                start=(kk == 0 and mp == 0 and xp == 0),
                                stop=(kk == DT - 1 and mp == 1),
                            )
                for c in range(SC):
                    split_psum(
                        pss[c],
                        qu[m][:, 0, c * CH : (c + 1) * CH],
                        qu[m][:, 1, c * CH : (c + 1) * CH],
                    )

        # ---- phase D: per q-tile attention, one-stage software pipeline:
        # PE runs scores(qi), then AV of qi-1 while ACT exponentiates qi and
        # the DMA xbar transposes qi's softmax weights. ----
        with (
            tc.tile_pool(name="stats", bufs=4) as p_st,
            tc.tile_pool(name="ssb", bufs=2) as p_ssb,
            tc.tile_pool(name="exps", bufs=2) as p_ex,
            tc.tile_pool(name="wtsb", bufs=2) as p_wtsb,
            tc.tile_pool(name="osb", bufs=2) as p_o,
            tc.tile_pool(name="scps", bufs=1, space="PSUM") as p_sc,
            tc.tile_pool(name="avps", bufs=1, space="PSUM") as p_av,
        ):

            def emit_scores(qi):
                ssb = p_ssb.tile([P, T], F32, tag="ssb")
                for c in range(TC):
                    scs[c] = p_sc.tile([P, CH], F32, tag=f"sc{c}", name=f"sc{c}")
                for kk in range(DT):
                    for qp, kp in ((0, 0), (0, 1), (1, 0)):
                        lhs = qu[kk][:, qp, qi * P : (qi + 1) * P]
                        for c in range(TC):
                            nc.tensor.matmul(
                                scs[c],
                                lhs,
                                xsp_u[kk][:, kp, c * CH : (c + 1) * CH],
                                start=(kk == 0 and qp == 0 and kp == 0),
                                stop=(kk == DT - 1 and qp == 1),
                            )
                for c in range(TC):
                    nc.vector.tensor_copy(
                        ssb[:, c * CH : (c + 1) * CH], scs[c]
                    )
                return ssb

            def emit_softmax(qi, ssb):
                mx4 = p_st.tile([P, TC], F32, tag="mx4")
                for c in range(TC):
                    nc.vector.reduce_max(
                        mx4[:, c : c + 1],
                        ssb[:, c * CH : (c + 1) * CH],
                        axis=mybir.AxisListType.X,
                    )
                negmx = p_st.tile([P, 1], F32, tag="negmx")
                mx = p_st.tile([P, 1], F32, tag="mx")
                nc.vector.reduce_max(mx, mx4, axis=mybir.AxisListType.X)
                nc.scalar.mul(negmx, mx, -1.0)
                sums = p_st.tile([P, TC], F32, tag="sums")
                exps = p_ex.tile([P, T], BF16, tag="exps")
                for c in range(TC):
                    nc.scalar.activation(
                        out=exps[:, c * CH : (c + 1) * CH],
                        in_=ssb[:, c * CH : (c + 1) * CH],
                        func=mybir.ActivationFunctionType.Exp,
                        bias=negmx[:, 0:1],
                        scale=1.0,
                        accum_out=sums[:, c : c + 1],
                    )
                ssum = p_st.tile([P, 1], F32, tag="ssum")
                nc.vector.reduce_sum(ssum, sums, axis=mybir.AxisListType.X)
                rsum = p_st.tile([P, 1], F32, tag="rsum")
                nc.vector.reciprocal(rsum, ssum)
                # full [s, t] -> [t, s] transpose on the DMA xbar:
                # wt[p, k, j] = exps[j, k*P + p]
                wt = p_wtsb.tile([P, TT, P], BF16, tag="wt")
                nc.sync.dma_start_transpose(wt[:], exps[:])
                return wt, rsum

            def emit_av(qi, wt, rsum):
                avs = [
                    p_av.tile([P, CH], F32, tag=f"av{n}", name=f"av{n}")
                    for n in range(DC)
                ]
                for t in range(TT):
                    lhs = wt[:, t, :]
                    for n in range(DC):
                        nc.tensor.matmul(
                            avs[n],
                            lhs,
                            v_sb[t][:, n * CH : (n + 1) * CH],
                            start=(t == 0),
                            stop=(t == TT - 1),
                        )
                osb = p_o.tile([P, D], F32, tag="o")
                for n in range(DC):
                    nc.vector.tensor_scalar_mul(
                        osb[:, n * CH : (n + 1) * CH], avs[n], rsum[:, 0:1]
                    )
                nc.scalar.dma_start(out=out_d[qi * P : (qi + 1) * P, :], in_=osb)

            scs = [None] * TC
            prev = None
            for qi in range(QT):
                ssb = emit_scores(qi)
                if prev is not None:
                    emit_av(*prev)
                wt, rsum = emit_softmax(qi, ssb)
                prev = (qi, wt, rsum)
            emit_av(*prev)

    nc.compile()
    return nc


_CACHE = {}


def _built_full():
    if "nc" not in _CACHE:
        _CACHE["nc"] = build_attention(1024, 2048, 1024)
    return _CACHE["nc"]


def _bf16_split(a):
    """fp32 array -> (hi, lo) bf16 with hi + lo ~= a (RNE, matches DVE)."""
    import ml_dtypes

    hi = a.astype(ml_dtypes.bfloat16)
    lo = (a - hi.astype(np.float32)).astype(ml_dtypes.bfloat16)
    return hi, lo


def host_prep_x(x_rows, P=128):
    """x rows [XR, D] f32 -> [DT, P, 2, XR] bf16 (x^T per d-tile, split)."""
    XR, D = x_rows.shape
    xT = np.ascontiguousarray(x_rows.T.astype(np.float32))  # [D, XR]
    hi, lo = _bf16_split(xT)
    out = np.stack([hi, lo], axis=1).reshape(D // P, P, 2, XR)
    return np.ascontiguousarray(out)


def host_prep_wT(w, c0=None, c1=None, P=128):
    """w [D, D] f32 -> w^T cols [c0:c1] as [DT, P, 2, c1-c0] bf16 hi/lo."""
    D = w.shape[0]
    wT = np.ascontiguousarray(w.astype(np.float32).T)  # [j, a]
    if c0 is not None:
        wT = wT[:, c0:c1]
    hi, lo = _bf16_split(wT)
    out = np.stack([hi, lo], axis=1).reshape(D // P, P, 2, wT.shape[1])
    return np.ascontiguousarray(out)


def host_prep_wv(wv, P=128):
    import ml_dtypes

    D = wv.shape[0]
    return np.ascontiguousarray(
        wv.astype(np.float32).astype(ml_dtypes.bfloat16).reshape(D // P, P, D)
    )


def _make_in_maps(x, wq, wk, wv):
    """Per-core input maps: core c = (batch c//2, query-half c%2)."""
    x = np.ascontiguousarray(np.asarray(x, dtype=np.float32))
    wq = np.asarray(wq, dtype=np.float32)
    wk = np.asarray(wk, dtype=np.float32)
    wv = np.asarray(wv, dtype=np.float32)
    B, S, D = x.shape
    half = S // 2
    wkt = host_prep_wT(wk)
    wvb = host_prep_wv(wv)
    in_maps = []
    for c in range(8):
        b, h = divmod(c, 2)
        if h == 0:
            xloc = x[b]
        else:
            xloc = np.concatenate([x[b][half:], x[b][:half]], axis=0)
        in_maps.append(
            {
                "xsp": host_prep_x(xloc),
                "wqt": host_prep_wT(wq, c * P, (c + 1) * P),
                "wkt": wkt,
                "wvb": wvb,
            }
        )
    return in_maps, (B, S, D)


def _assemble(results, shape):
    B, S, D = shape
    half = S // 2
    out = np.empty((B, S, D), np.float32)
    for c in range(8):
        b, h = divmod(c, 2)
        out[b, h * half : (h + 1) * half] = results[c]["out"]
    return out


def kernel(x, wq, wk, wv):
    """Full (unsharded) inputs -> full output, running SPMD on 8 cores."""
    from concourse.bass_utils import run_bass_kernel_spmd

    in_maps, shape = _make_in_maps(x, wq, wk, wv)
    nc = _built_full()
    res = run_bass_kernel_spmd(nc, in_maps, core_ids=list(range(8))).results
    return _assemble(res, shape)
